# revision 22
# baseline (speedup 1.0000x reference)
"""Trainium2 Bass kernel for nn_AxialBlock (3-axis axial attention sum).

Problem (hardcoded): x (B=4, C=512, T=16, H=32, W=32) fp32, three axial
MHA blocks (attend along W, H, T; n_head=8, d=64) each with their own
QKVO projections; outputs summed. Output (B, C, T, H, W) fp32.

Sharding: 8 cores = (batch b in 0..3) x (pair index j in 0..1).
  - w-pass / t-pass: tokens split by H-half (j); fully local.
  - h-pass: head-parallel within the pair: each core computes heads
    4j..4j+4 over the FULL sample (attention along H needs all H), and
    writes a PARTIAL y_h (its heads' out-projection contribution) for
    all 16384 tokens; the host sums the two partials per sample. This
    replaces the baseline's k/v full recompute (-33% h-pass matmul work).

On-device layout: x is channels-first ("x^T", C on partitions). Host
pre-permutes x into three token orders (w-fastest / t-fastest /
h-fastest) so each axial attention acts on 32 consecutive tokens.

Matmul structure: all projection matmuls (q/k/out-proj) are emitted in
super-tiles of ST=4 token tiles with the weight chunk as the stationary
operand reused across the 4 sub-tiles (4 PSUM banks accumulate in
parallel) — the PE reloads its stationary every matmul otherwise, and
the ~107ns weight load is NOT hidden (measured 1.7x on a microbench).
v must be token-partitioned (it is the O^T stationary), so its
projection keeps per-tile stationaries.

Attention per 512-token tile (16 rows x 32 tokens): k is evacuated
parity-split into persistent pre-zeroed "kz" buffers (one head per 64
d-rows) so scores contract over all 128 partitions; one (K=128, M=32,
N=64) matmul per (chunk, row) computes both heads of the chunk at
col-tile (0, 32j). Softmax: exp on ScalarE, reduce+reciprocal on
VectorE, broadcast normalize on GpSimd. The t-pass cross-fiber mask is
a rank-2 matmul (-60 additive) accumulated under the scores before exp.
A -> A^T via the DVE 32x32 block transpose, then DVE copies form a
block-diagonal A^T ("abd"); o^T = V^T @ abd lands feature-partitioned;
then the out-projection (ST=4 weight reuse) and y accumulation: w-pass
writes y + summed bias, t-pass does a strided DRAM read-modify-write
add, h-pass writes its own partial y_h (bf16) with no rmw.
"""

import contextlib

import ml_dtypes
import numpy as np

import concourse.bass as bass
import concourse.tile as tile
from concourse import bacc, mybir
from concourse.bass_utils import run_bass_kernel_spmd

BF16 = mybir.dt.bfloat16
FP32 = mybir.dt.float32
BF16_NP = np.dtype(ml_dtypes.bfloat16)

B, C, T, H, W = 4, 512, 16, 32, 32
NH, D = 8, 64
HL = H // 2              # per-core H slice (w/t passes)
N_CORES = 8
TOK_LOCAL = T * HL * W   # 8192 tokens owned per core (w/t)
TOK_FULL = T * H * W     # 16384 tokens in a batch sample (h)
TILE = 512               # tokens per on-chip tile
NCH = C // 128           # 4 partition chunks of the feature dim
NHL = NH // 2            # 4 local heads in the h-pass
CL = NHL * D             # 256 local feature width in the h-pass
NCHL = CL // 128         # 2 chunks of local features
ST = 4                   # sub-tiles per super-tile (stationary reuse)


def _proj_phase(tc, ps_pool, n_mc, n_kc, lhs_fn, rhs_fn, evac_fn, width=TILE):
    """One ST-wide projection phase: stationary reused across ST sub-tiles.

    lhs_fn(mc, kc) -> stationary AP; rhs_fn(st, kc) -> moving AP;
    evac_fn(mc, st, ps) consumes the finished PSUM tile.
    """
    nc = tc.nc
    for mc in range(n_mc):
        pss = []
        for st in range(ST):
            ps = ps_pool.tile([128, TILE], FP32, tag=f"p{st}", bufs=2,
                              name=f"ps{st}")
            pss.append(ps)
        for kc in range(n_kc):
            for st in range(ST):
                nc.tensor.matmul(
                    pss[st][0:128, 0:width],
                    lhsT=lhs_fn(mc, kc),
                    rhs=rhs_fn(st, kc),
                    start=(kc == 0), stop=(kc == n_kc - 1),
                )
        for st in range(ST):
            evac_fn(mc, st, pss[st])


def _attention(tc, pools, axis, st, q_sb, kz_sb, v_sb, ot_sb, tml_sb, tmr_sb,
               abd_tiles, nch_q, nhd):
    """S + softmax + O^T for one 512-token sub-tile.

    nch_q: feature chunks of q/k (4 for w/t, 2 for h); nhd: heads (8 or 4).
    Scores psum free layout: w/t: two banks of (2 row-groups x 8 heads x 32);
    h: one bank of (4 row-groups x 4 heads x 32).
    """
    nc = tc.nc
    (xt_pool, qk_pool, v_pool, a_pool, sm_pool,
     ot_pool, y_pool, ps_pool) = pools
    GW = nhd * 32
    n_bank_groups = 512 // (2 * GW) if nhd == NH else 1  # 2-rowgroup banks

    abd_by_g = {}
    if nhd == NH:
        # w/t: 2 psum banks, each covering 2 row-groups (gh)
        for gg in range(2):
            sps = ps_pool.tile([128, 2 * GW], FP32, tag=f"p{gg}", bufs=2,
                               name="sps")
            base = axis == "t"
            if base:
                nc.tensor.matmul(
                    sps[:], lhsT=tml_sb[:], rhs=tmr_sb[:],
                    start=True, stop=False, skip_group_check=True,
                )
            nmm = 32
            i_mm = 0
            for gh in range(2):
                g = 2 * gg + gh
                for c in range(nch_q):
                    for j in range(4):
                        qcol = (g * 4 + j) * 32
                        i_mm += 1
                        nc.tensor.matmul(
                            sps[32 * j:32 * (j + 1),
                                gh * GW + 2 * c * 32:gh * GW + (2 * c + 2) * 32],
                            lhsT=q_sb[:, st, c, qcol:qcol + 32],
                            rhs=kz_sb[:, :, c,
                                      (g * 4 + j) * 32:(g * 4 + j) * 32 + 32],
                            tile_position=(0, 32 * j),
                            start=(not base),
                            stop=(base and i_mm == nmm),
                            skip_group_check=True,
                        )
            # softmax over k (free axis), one op per step per 2 row-groups
            a_sb = a_pool.tile([128, 2 * GW], BF16, tag="a")
            nc.scalar.activation(a_sb[:], sps[:],
                                 mybir.ActivationFunctionType.Exp)
            a3 = a_sb[:].rearrange("p (n k) -> p n k", n=2 * nhd)
            sums = sm_pool.tile([128, 2 * nhd], FP32, tag="sums")
            nc.vector.tensor_reduce(
                sums[:], a3, axis=mybir.AxisListType.X, op=mybir.AluOpType.add
            )
            recip = sm_pool.tile([128, 2 * nhd], FP32, tag="recip")
            nc.vector.reciprocal(recip[:], sums[:])
            nc.gpsimd.tensor_tensor(
                a3, a3,
                recip[:].unsqueeze(2).broadcast_to((128, 2 * nhd, 32)),
                mybir.AluOpType.mult,
            )
            at_sb = a_pool.tile([128, 2 * GW], BF16, tag="at")
            nc.vector.transpose(at_sb[:], a_sb[:])
            abd = abd_tiles[tc._abd_flip]
            tc._abd_flip ^= 1
            for j in range(4):
                nc.vector.tensor_copy(
                    abd[32 * j:32 * (j + 1), 512 * j:512 * (j + 1)],
                    at_sb[32 * j:32 * (j + 1), :],
                )
            abd_by_g[2 * gg] = abd
            abd_by_g[2 * gg + 1] = abd

        # O^T = V^T @ A_bd, chunk-outer
        for c in range(nch_q):
            otp = ps_pool.tile([128, TILE], FP32, name="otp",
                               tag=f"p{2 + c % 2}", bufs=2)
            for g in range(4):
                gh = g % 2
                abd4 = abd_by_g[g][:].rearrange("p (j x) -> p j x", j=4)
                for p in range(2):
                    s0 = gh * GW + (2 * c + p) * 32
                    nc.tensor.matmul(
                        otp[64 * p:64 * (p + 1), g * 128:(g + 1) * 128],
                        lhsT=v_sb[:, g, (2 * c + p) * 64:(2 * c + p + 1) * 64],
                        rhs=abd4[:, :, s0:s0 + 32],
                        tile_position=(0, 64 * p),
                    )
            if c % 2 == 0:
                nc.scalar.copy(ot_sb[:, st, c, :], otp[:])
            else:
                nc.vector.tensor_copy(ot_sb[:, st, c, :], otp[:])
    else:
        # h-pass (4 local heads): all 4 row-groups in ONE psum bank,
        # free = g*128 + head*32 + kpos; no mask, full 32-token rows.
        sps = ps_pool.tile([128, 4 * GW], FP32, tag=f"p{st % 2}", bufs=2,
                           name="sps")
        for g in range(4):
            for c in range(nch_q):
                for j in range(4):
                    qcol = (g * 4 + j) * 32
                    nc.tensor.matmul(
                        sps[32 * j:32 * (j + 1),
                            g * GW + 2 * c * 32:g * GW + (2 * c + 2) * 32],
                        lhsT=q_sb[:, st, c, qcol:qcol + 32],
                        rhs=kz_sb[:, :, c,
                                  (g * 4 + j) * 32:(g * 4 + j) * 32 + 32],
                        tile_position=(0, 32 * j),
                        start=True, stop=False,
                        skip_group_check=True,
                    )
        a_sb = a_pool.tile([128, 4 * GW], BF16, tag="a")
        nc.scalar.activation(a_sb[:], sps[:],
                             mybir.ActivationFunctionType.Exp)
        a3 = a_sb[:].rearrange("p (n k) -> p n k", n=4 * nhd)
        sums = sm_pool.tile([128, 4 * nhd], FP32, tag="sums")
        nc.vector.tensor_reduce(
            sums[:], a3, axis=mybir.AxisListType.X, op=mybir.AluOpType.add
        )
        recip = sm_pool.tile([128, 4 * nhd], FP32, tag="recip")
        nc.vector.reciprocal(recip[:], sums[:])
        nc.gpsimd.tensor_tensor(
            a3, a3,
            recip[:].unsqueeze(2).broadcast_to((128, 4 * nhd, 32)),
            mybir.AluOpType.mult,
        )
        at_sb = a_pool.tile([128, 4 * GW], BF16, tag="at")
        nc.vector.transpose(at_sb[:], a_sb[:])
        abd = abd_tiles[tc._abd_flip]
        tc._abd_flip ^= 1
        # at[32j+k, g*128 + h*32 + q] -> abd[32j+k, g*512 + j*128 + h*32 + q]
        for j in range(4):
            src = at_sb[32 * j:32 * (j + 1), :].rearrange(
                "p (g x) -> p g x", g=4)
            dst = abd[32 * j:32 * (j + 1), :].rearrange(
                "p (g x) -> p g x", g=4)[:, :, 128 * j:128 * (j + 1)]
            nc.vector.tensor_copy(dst, src)
        for c in range(NCHL):
            otp = ps_pool.tile([128, TILE], FP32, name="otp",
                               tag=f"p{2 + c % 2}", bufs=2)
            for g in range(4):
                abd4 = abd[:, 512 * g:512 * (g + 1)].rearrange(
                    "p (j x) -> p j x", j=4)
                for p in range(2):
                    s0 = (2 * c + p) * 32
                    nc.tensor.matmul(
                        otp[64 * p:64 * (p + 1), g * 128:(g + 1) * 128],
                        lhsT=v_sb[:, g, (2 * c + p) * 64:(2 * c + p + 1) * 64],
                        rhs=abd4[:, :, s0:s0 + 32],
                        tile_position=(0, 64 * p),
                    )
            if c % 2 == 0:
                nc.scalar.copy(ot_sb[:, st, c, :], otp[:])
            else:
                nc.vector.tensor_copy(ot_sb[:, st, c, :], otp[:])


def _build_pass_wt(tc, pools, axis, x_ap, w_aps, y_ap, bias_aps, tml_sb,
                   tmr_sb, kz_tiles, abd_tiles):
    """w- or t-axis pass over the core's 8192 owned tokens."""
    nc = tc.nc
    wq_sb, wk_sb, wv_sb, wo_sb = w_aps
    (xt_pool, qk_pool, v_pool, a_pool, sm_pool,
     ot_pool, y_pool, ps_pool) = pools
    ntiles = TOK_LOCAL // TILE
    y4d = y_ap.rearrange("c (t h w) -> c t h w", t=T, h=HL, w=W)

    for sup in range(ntiles // ST):
        it0 = sup * ST
        xt = xt_pool.tile([128, ST, NCH, TILE], BF16, tag="xt")
        for st in range(ST):
            for kc in range(NCH):
                nc.sync.dma_start(
                    xt[:, st, kc, :],
                    x_ap[128 * kc:128 * (kc + 1),
                         (it0 + st) * TILE:(it0 + st + 1) * TILE])

        # ---- q^T projection (feat-partition), stationary reused over ST
        q_sb = qk_pool.tile([128, ST, NCH, TILE], BF16, tag="q", bufs=1)

        def q_evac(mc, st, ps):
            if st % 2 == 0:
                nc.scalar.copy(q_sb[:, st, mc, :], ps[:])
            else:
                nc.vector.tensor_copy(q_sb[:, st, mc, :], ps[:])

        _proj_phase(tc, ps_pool, NCH, NCH,
                    lambda mc, kc: wq_sb[:, kc, 128 * mc:128 * (mc + 1)],
                    lambda st, kc: xt[:, st, kc, :], q_evac)

        # ---- k^T projection, evacuated parity-split into kz buffers
        def k_evac(mc, st, ps):
            kz = kz_tiles[st]
            if mc < 2:
                nc.scalar.copy(kz[0:64, 0, mc, :], ps[0:64, :])
                nc.scalar.copy(kz[64:128, 1, mc, :], ps[64:128, :])
            else:
                nc.vector.tensor_copy(kz[0:64, 0, mc, :], ps[0:64, :])
                nc.vector.tensor_copy(kz[64:128, 1, mc, :], ps[64:128, :])

        _proj_phase(tc, ps_pool, NCH, NCH,
                    lambda mc, kc: wk_sb[:, kc, 128 * mc:128 * (mc + 1)],
                    lambda st, kc: xt[:, st, kc, :], k_evac)

        # ---- v projection (token-partition, stationary = x) interleaved
        # with attention per sub-tile so dense v matmuls fill the PE while
        # the previous sub-tile's softmax chain runs on Scalar/Vector/GpSimd
        ot_sb = ot_pool.tile([128, ST, NCH, TILE], BF16, tag="ot", bufs=1)
        for st in range(ST):
            v_sb = v_pool.tile([128, NCH, C], BF16, tag=f"v{st}", bufs=1,
                               name=f"v{st}")
            pss = []
            for ts in range(NCH):
                ps = ps_pool.tile([128, TILE], FP32, tag=f"p{ts}", bufs=2,
                                  name="psv")
                pss.append(ps)
            for kc in range(NCH):
                for ts in range(NCH):
                    nc.tensor.matmul(
                        pss[ts][:],
                        lhsT=xt[:, st, kc, 128 * ts:128 * (ts + 1)],
                        rhs=wv_sb[:, kc, :],
                        start=(kc == 0), stop=(kc == NCH - 1),
                    )
            for ts in range(NCH):
                if ts % 2 == 0:
                    nc.scalar.copy(v_sb[:, ts, :], pss[ts][:])
                else:
                    nc.vector.tensor_copy(v_sb[:, ts, :], pss[ts][:])
            _attention(tc, pools, axis, st, q_sb, kz_tiles[st], v_sb,
                       ot_sb, tml_sb, tmr_sb, abd_tiles, NCH, NH)

        # ---- out-projection (stationary reused over ST) + y accumulate
        def y_evac(mc, st, ps):
            it = it0 + st
            cs = slice(128 * mc, 128 * (mc + 1))
            if axis == "w":
                y_sb = y_pool.tile([128, TILE], FP32, tag="yw")
                nc.scalar.activation(
                    y_sb[:], ps[:], mybir.ActivationFunctionType.Identity,
                    bias=bias_aps[mc],
                )
                nc.sync.dma_start(y_ap[cs, it * TILE:(it + 1) * TILE], y_sb[:])
            else:
                # tile it covers h-row `it`; psum tokens are (w 32, t 16).
                # Evacuate transposed to (t, w), then accumulate into y via
                # a software-DGE DMA with on-the-fly add (no read-back, and
                # the psum bank frees after the copy, not the whole rmw).
                y_slice = y4d[cs, :, it, :]                   # (128, t16, w32)
                ynew = y_pool.tile([128, T, W], FP32, tag="yt2")
                yp3 = ps[:].rearrange("p (w t) -> p w t", w=W).transpose(
                    [0, 2, 1])
                if st % 2 == 0:
                    nc.scalar.copy(ynew[:], yp3)
                else:
                    nc.vector.tensor_copy(ynew[:], yp3)
                nc.gpsimd.dma_start(y_slice, ynew[:],
                                    accum_op=mybir.AluOpType.add)

        _proj_phase(tc, ps_pool, NCH, NCH,
                    lambda mc, kc: wo_sb[:, kc, 128 * mc:128 * (mc + 1)],
                    lambda st, kc: ot_sb[:, st, kc, :], y_evac)


def _build_pass_h(tc, pools, x_ap, w_aps, yh_ap, kz_tiles, abd_tiles):
    """h-axis pass: this core's 4 heads over the FULL sample; partial y_h."""
    nc = tc.nc
    wq_sb, wk_sb, wv_sb, wo_sb = w_aps
    (xt_pool, qk_pool, v_pool, a_pool, sm_pool,
     ot_pool, y_pool, ps_pool) = pools
    ntiles = TOK_FULL // TILE

    # The w/t passes leave a different nonzero footprint in abd (full
    # 512-wide strips); the h-pass layout assumes zeros outside its own
    # g*512+j*128 blocks, so re-zero before reuse.
    for t in abd_tiles:
        nc.gpsimd.memset(t[:], 0.0)

    for sup in range(ntiles // ST):
        it0 = sup * ST
        xt = xt_pool.tile([128, ST, NCH, TILE], BF16, tag="xt")
        for st in range(ST):
            for kc in range(NCH):
                nc.sync.dma_start(
                    xt[:, st, kc, :],
                    x_ap[128 * kc:128 * (kc + 1),
                         (it0 + st) * TILE:(it0 + st + 1) * TILE])

        q_sb = qk_pool.tile([128, ST, NCH, TILE], BF16, tag="q", bufs=1)

        def q_evac(mc, st, ps):
            if st % 2 == 0:
                nc.scalar.copy(q_sb[:, st, mc, :], ps[:])
            else:
                nc.vector.tensor_copy(q_sb[:, st, mc, :], ps[:])

        _proj_phase(tc, ps_pool, NCHL, NCH,
                    lambda mc, kc: wq_sb[:, kc, 128 * mc:128 * (mc + 1)],
                    lambda st, kc: xt[:, st, kc, :], q_evac)

        def k_evac(mc, st, ps):
            kz = kz_tiles[st]
            if mc == 0:
                nc.scalar.copy(kz[0:64, 0, mc, :], ps[0:64, :])
                nc.scalar.copy(kz[64:128, 1, mc, :], ps[64:128, :])
            else:
                nc.vector.tensor_copy(kz[0:64, 0, mc, :], ps[0:64, :])
                nc.vector.tensor_copy(kz[64:128, 1, mc, :], ps[64:128, :])

        _proj_phase(tc, ps_pool, NCHL, NCH,
                    lambda mc, kc: wk_sb[:, kc, 128 * mc:128 * (mc + 1)],
                    lambda st, kc: xt[:, st, kc, :], k_evac)

        ot_sb = ot_pool.tile([128, ST, NCH, TILE], BF16, tag="ot", bufs=1)
        for st in range(ST):
            v_sb = v_pool.tile([128, NCH, C], BF16, tag=f"v{st}", bufs=1,
                               name=f"v{st}")
            pss = []
            for ts in range(NCH):
                ps = ps_pool.tile([128, TILE], FP32, tag=f"p{ts}", bufs=2,
                                  name="psv")
                pss.append(ps)
            for kc in range(NCH):
                for ts in range(NCH):
                    nc.tensor.matmul(
                        pss[ts][0:128, 0:CL],
                        lhsT=xt[:, st, kc, 128 * ts:128 * (ts + 1)],
                        rhs=wv_sb[:, kc, 0:CL],
                        start=(kc == 0), stop=(kc == NCH - 1),
                    )
            for ts in range(NCH):
                if ts % 2 == 0:
                    nc.scalar.copy(v_sb[:, ts, 0:CL], pss[ts][0:128, 0:CL])
                else:
                    nc.vector.tensor_copy(v_sb[:, ts, 0:CL],
                                          pss[ts][0:128, 0:CL])
            _attention(tc, pools, "h", st, q_sb, kz_tiles[st], v_sb,
                       ot_sb, None, None, abd_tiles, NCHL, NHL)

        def y_evac(mc, st, ps):
            it = it0 + st
            cs = slice(128 * mc, 128 * (mc + 1))
            y_sb = y_pool.tile([128, TILE], BF16, tag="yh_sb")
            if mc % 2 == 0:
                nc.scalar.copy(y_sb[:], ps[:])
            else:
                nc.vector.tensor_copy(y_sb[:], ps[:])
            nc.sync.dma_start(yh_ap[cs, it * TILE:(it + 1) * TILE], y_sb[:])

        _proj_phase(tc, ps_pool, NCH, NCHL,
                    lambda mc, kc: wo_sb[:, kc, 128 * mc:128 * (mc + 1)],
                    lambda st, kc: ot_sb[:, st, kc, :], y_evac)


def build_program():
    """Build + compile the SPMD bass program (same program on all 8 cores)."""
    nc = bacc.Bacc(
        "TRN2", target_bir_lowering=False, debug=False,
        enable_asserts=False, num_devices=N_CORES,
    )

    def din(name, shape, dt=BF16):
        return nc.dram_tensor(name, shape, dt, kind="ExternalInput").ap()

    x_w = din("x_w", (C, TOK_LOCAL))
    x_t = din("x_t", (C, TOK_LOCAL))
    x_h = din("x_h", (C, TOK_FULL))
    w_in = {}
    for ax, cout in (("w", C), ("t", C)):
        for nm in ("wq", "wk", "wv", "wo"):
            w_in[f"{nm}_{ax}"] = din(f"{nm}_{ax}", (C, cout))
    for nm in ("wq", "wk", "wv"):
        w_in[f"{nm}_h"] = din(f"{nm}_h", (C, CL))
    w_in["wo_h"] = din("wo_h", (CL, C))
    bias_in = din("bias", (C, 1), FP32)
    tml_in = din("tml", (2, 128))
    tmr_in = din("tmr", (2, 512))
    y_ap = nc.dram_tensor("y", (C, TOK_LOCAL), FP32, kind="ExternalOutput").ap()
    yh_ap = nc.dram_tensor("y_h", (C, TOK_FULL), BF16,
                           kind="ExternalOutput").ap()

    with tile.TileContext(nc) as tc:
        with contextlib.ExitStack() as ctx:
            xt_pool = ctx.enter_context(tc.tile_pool(name="xt", bufs=2))
            w_pool = ctx.enter_context(tc.tile_pool(name="wts", bufs=2))
            qk_pool = ctx.enter_context(tc.tile_pool(name="qk", bufs=1))
            v_pool = ctx.enter_context(tc.tile_pool(name="v", bufs=1))
            a_pool = ctx.enter_context(tc.tile_pool(name="a", bufs=3))
            sm_pool = ctx.enter_context(tc.tile_pool(name="sm", bufs=3))
            ot_pool = ctx.enter_context(tc.tile_pool(name="ot", bufs=1))
            y_pool = ctx.enter_context(tc.tile_pool(name="y", bufs=3))
            ps_pool = ctx.enter_context(tc.tile_pool(name="ps", bufs=1,
                                                     space="PSUM"))
            const_pool = ctx.enter_context(tc.tile_pool(name="const", bufs=1))

            # constants
            tml_sb = const_pool.tile([2, 128], BF16)
            nc.sync.dma_start(tml_sb[:], tml_in[:])
            tmr_sb = const_pool.tile([2, 512], BF16)
            nc.sync.dma_start(tmr_sb[:], tmr_in[:])
            bias_sb = const_pool.tile([128, NCH], FP32)
            for mc in range(NCH):
                nc.sync.dma_start(
                    bias_sb[:, mc:mc + 1], bias_in[128 * mc:128 * (mc + 1), :]
                )
            bias_aps = [bias_sb[:, mc:mc + 1] for mc in range(NCH)]

            # persistent block-diagonal A^T buffers and parity-split k
            # buffers (one per sub-tile), zeroed once
            abd_tiles = []
            for i in range(2):
                t = const_pool.tile([128, 4 * 512], BF16, name=f"abd{i}")
                nc.gpsimd.memset(t[:], 0.0)
                abd_tiles.append(t)
            tc._abd_flip = 0
            kz_tiles = []
            for i in range(ST):
                t = const_pool.tile([128, 2, NCH, TILE], BF16, name=f"kz{i}")
                nc.gpsimd.memset(t[:], 0.0)
                kz_tiles.append(t)

            pools = (xt_pool, qk_pool, v_pool, a_pool, sm_pool,
                     ot_pool, y_pool, ps_pool)

            def load_w(ax, shapes):
                w_aps = []
                for nm, n_kc, ncol in shapes:
                    wt = w_pool.tile([128, NCH, C], BF16, tag=nm, name=nm)
                    for kc in range(n_kc):
                        nc.sync.dma_start(
                            wt[:, kc, 0:ncol],
                            w_in[f"{nm}_{ax}"][128 * kc:128 * (kc + 1), :],
                        )
                    w_aps.append(wt)
                return w_aps

            wt_shapes = [("wq", NCH, C), ("wk", NCH, C), ("wv", NCH, C),
                         ("wo", NCH, C)]
            h_shapes = [("wq", NCH, CL), ("wk", NCH, CL), ("wv", NCH, CL),
                        ("wo", NCHL, C)]

            w_aps = load_w("w", wt_shapes)
            _build_pass_wt(tc, pools, "w", x_w, w_aps, y_ap, bias_aps,
                           tml_sb, tmr_sb, kz_tiles, abd_tiles)
            w_aps = load_w("t", wt_shapes)
            _build_pass_wt(tc, pools, "t", x_t, w_aps, y_ap, bias_aps,
                           tml_sb, tmr_sb, kz_tiles, abd_tiles)
            w_aps = load_w("h", h_shapes)
            _build_pass_h(tc, pools, x_h, w_aps, yh_ap, kz_tiles, abd_tiles)

    nc.compile()
    return nc


_PROGRAM = None


def _get_program():
    global _PROGRAM
    if _PROGRAM is None:
        _PROGRAM = build_program()
    return _PROGRAM


def make_in_maps(inputs):
    """Host-side shard + layout prep: per-core input dicts."""
    x = np.asarray(inputs["x"], np.float32)          # (B, C, T, H, W)
    scale = 1.0 / np.sqrt(D)

    weights = {}
    for ax in ("w", "t"):
        for nm in ("wq", "wk", "wv", "wo"):
            wm = np.asarray(inputs[f"{nm}_{ax}"], np.float32)
            if nm == "wq":
                wm = wm * scale
            # lhsT layout: (C_in, C_out) = W.T
            weights[f"{nm}_{ax}"] = np.ascontiguousarray(wm.T).astype(BF16_NP)
    # h-pass: per-pair-half head slices
    h_w = {}
    for half in range(2):
        cols = slice(CL * half, CL * (half + 1))
        m = {}
        for nm in ("wq", "wk", "wv"):
            wm = np.asarray(inputs[f"{nm}_h"], np.float32)
            if nm == "wq":
                wm = wm * scale
            m[f"{nm}_h"] = np.ascontiguousarray(wm.T[:, cols]).astype(BF16_NP)
        wo = np.asarray(inputs["wo_h"], np.float32)
        m["wo_h"] = np.ascontiguousarray(wo.T[cols, :]).astype(BF16_NP)
        h_w[half] = m
    bias = (np.asarray(inputs["bo_w"], np.float32)
            + np.asarray(inputs["bo_h"], np.float32)
            + np.asarray(inputs["bo_t"], np.float32)).reshape(C, 1)

    # rank-2 additive cross-fiber mask for the t-pass:
    # S += tml.T @ tmr with tml one-hot on the query fiber and tmr = -60 on
    # cross-fiber key columns
    p = np.arange(128) % 32
    tml = np.stack([(p // 16) == e for e in range(2)]).astype(BF16_NP)
    f = np.arange(512) % 32
    tmr = np.stack([np.where((f // 16) != e, -60.0, 0.0) for e in range(2)]
                   ).astype(BF16_NP)

    in_maps = []
    for core in range(N_CORES):
        b, j = divmod(core, 2)
        xb = x[b]                                    # (C, T, H, W)
        xw = xb[:, :, 16 * j:16 * (j + 1), :]        # (C, T, HL, W) w-fastest
        xt = np.transpose(xw, (0, 2, 3, 1))          # (C, HL, W, T) t-fastest
        xh = np.transpose(xb, (0, 1, 3, 2))          # (C, T, W, H) h-fastest
        m = {
            "x_w": np.ascontiguousarray(xw).reshape(C, TOK_LOCAL).astype(BF16_NP),
            "x_t": np.ascontiguousarray(xt).reshape(C, TOK_LOCAL).astype(BF16_NP),
            "x_h": np.ascontiguousarray(xh).reshape(C, TOK_FULL).astype(BF16_NP),
            "bias": bias, "tml": tml, "tmr": tmr,
        }
        m.update(weights)
        m.update(h_w[j])
        in_maps.append(m)
    return in_maps


def assemble_output(results):
    """Gather per-core y/y_h into (B, C, T, H, W) fp32."""
    out = np.empty((B, C, T, H, W), np.float32)
    for b in range(B):
        c0, c1 = 2 * b, 2 * b + 1
        out[b, :, :, 0:HL, :] = np.asarray(results[c0]["y"]).reshape(
            C, T, HL, W)
        out[b, :, :, HL:H, :] = np.asarray(results[c1]["y"]).reshape(
            C, T, HL, W)
        yh = (np.asarray(results[c0]["y_h"]).astype(np.float32)
              + np.asarray(results[c1]["y_h"]).astype(np.float32))
        out[b] += yh.reshape(C, T, W, H).transpose(0, 1, 3, 2)
    return out


_RUNNER = None


def _get_runner():
    """Build the sharded PJRT callable once; reuse across kernel() calls."""
    global _RUNNER
    if _RUNNER is not None:
        return _RUNNER
    import jax
    from jax.sharding import Mesh, PartitionSpec
    from jax.experimental.shard_map import shard_map
    from concourse import bass2jax

    nc = _get_program()
    bass2jax.install_neuronx_cc_hook()
    partition_name = (nc.partition_id_tensor.name
                      if nc.partition_id_tensor else None)
    in_names, out_names, out_avals, zero_outs = [], [], [], []
    for alloc in nc.m.functions[0].allocations:
        if not isinstance(alloc, mybir.MemoryLocationSet):
            continue
        name = alloc.memorylocations[0].name
        if alloc.kind == "ExternalInput":
            if name != partition_name:
                in_names.append(name)
        elif alloc.kind == "ExternalOutput":
            out_names.append(name)
            shape = tuple(alloc.tensor_shape)
            dtype = mybir.dt.np(alloc.dtype)
            out_avals.append(jax.core.ShapedArray(shape, dtype))
            zero_outs.append(np.zeros((N_CORES * shape[0], *shape[1:]), dtype))
    n_params = len(in_names)
    all_in_names = list(in_names) + out_names
    if partition_name is not None:
        all_in_names.append(partition_name)

    def _body(*args):
        operands = list(args)
        if partition_name is not None:
            operands.append(bass2jax.partition_id_tensor())
        return tuple(bass2jax._bass_exec_p.bind(
            *operands,
            out_avals=tuple(out_avals),
            in_names=tuple(all_in_names),
            out_names=tuple(out_names),
            lowering_input_output_aliases=(),
            sim_require_finite=True,
            sim_require_nnan=True,
            nc=nc,
        ))

    devices = jax.devices()[:N_CORES]
    mesh = Mesh(np.asarray(devices), ("core",))
    in_specs = (PartitionSpec("core"),) * (n_params + len(out_names))
    out_specs = (PartitionSpec("core"),) * len(out_names)
    fn = jax.jit(shard_map(_body, mesh=mesh, in_specs=in_specs,
                           out_specs=out_specs, check_rep=False))

    def run(in_maps):
        concat_in = [
            np.concatenate([np.asarray(in_maps[c][nm]) for c in range(N_CORES)],
                           axis=0)
            for nm in in_names
        ]
        outs = fn(*concat_in, *zero_outs)
        return [
            {nm: np.asarray(outs[i]).reshape(N_CORES, *out_avals[i].shape)[c]
             for i, nm in enumerate(out_names)}
            for c in range(N_CORES)
        ]

    _RUNNER = run
    return run


def kernel(**inputs) -> np.ndarray:
    run = _get_runner()
    in_maps = make_in_maps(inputs)
    return assemble_output(run(in_maps))


# revision 23
# speedup vs baseline: 1.0485x; 1.0485x over previous
"""Trainium2 Bass kernel for nn_AxialBlock (3-axis axial attention sum).

Problem (hardcoded): x (B=4, C=512, T=16, H=32, W=32) fp32, three axial
MHA blocks (attend along W, H, T; n_head=8, d=64) each with their own
QKVO projections; outputs summed. Output (B, C, T, H, W) fp32.

Sharding: 8 cores = (batch b in 0..3) x (pair index j in 0..1).
  - w-pass / t-pass: tokens split by H-half (j); fully local.
  - h-pass: head-parallel within the pair: each core computes heads
    4j..4j+4 over the FULL sample (attention along H needs all H), and
    writes a PARTIAL y_h (its heads' out-projection contribution) for
    all 16384 tokens; the host sums the two partials per sample. This
    replaces the baseline's k/v full recompute (-33% h-pass matmul work).

On-device layout: x is channels-first ("x^T", C on partitions). Host
pre-permutes x into three token orders (w-fastest / t-fastest /
h-fastest) so each axial attention acts on 32 consecutive tokens.

Matmul structure: all projection matmuls (q/k/out-proj) are emitted in
super-tiles of ST=4 token tiles with the weight chunk as the stationary
operand reused across the 4 sub-tiles (4 PSUM banks accumulate in
parallel) — the PE reloads its stationary every matmul otherwise, and
the ~107ns weight load is NOT hidden (measured 1.7x on a microbench).
v must be token-partitioned (it is the O^T stationary), so its
projection keeps per-tile stationaries.

Attention per 512-token tile (16 rows x 32 tokens): k is evacuated
parity-split into persistent pre-zeroed "kz" buffers (one head per 64
d-rows) so scores contract over all 128 partitions; one (K=128, M=32,
N=64) matmul per (chunk, row) computes both heads of the chunk at
col-tile (0, 32j). Softmax: exp on ScalarE, reduce+reciprocal on
VectorE, broadcast normalize on GpSimd. The t-pass cross-fiber mask is
a rank-2 matmul (-60 additive) accumulated under the scores before exp.
A -> A^T via the DVE 32x32 block transpose, then DVE copies form a
block-diagonal A^T ("abd"); o^T = V^T @ abd lands feature-partitioned;
then the out-projection (ST=4 weight reuse) and y accumulation: w-pass
writes y + summed bias, t-pass does a strided DRAM read-modify-write
add, h-pass writes its own partial y_h (bf16) with no rmw.
"""

import contextlib

import ml_dtypes
import numpy as np

import concourse.bass as bass
import concourse.tile as tile
from concourse import bacc, mybir
from concourse.bass_utils import run_bass_kernel_spmd

BF16 = mybir.dt.bfloat16
FP32 = mybir.dt.float32
BF16_NP = np.dtype(ml_dtypes.bfloat16)

B, C, T, H, W = 4, 512, 16, 32, 32
NH, D = 8, 64
HL = H // 2              # per-core H slice (w/t passes)
N_CORES = 8
TOK_LOCAL = T * HL * W   # 8192 tokens owned per core (w/t)
TOK_FULL = T * H * W     # 16384 tokens in a batch sample (h)
TILE = 512               # tokens per on-chip tile
NCH = C // 128           # 4 partition chunks of the feature dim
NHL = NH // 2            # 4 local heads in the h-pass
CL = NHL * D             # 256 local feature width in the h-pass
NCHL = CL // 128         # 2 chunks of local features
ST = 4                   # sub-tiles per super-tile (stationary reuse)


def _proj_phase(tc, ps_pool, n_mc, n_kc, lhs_fn, rhs_fn, evac_fn, width=TILE):
    """One ST-wide projection phase: stationary reused across ST sub-tiles.

    lhs_fn(mc, kc) -> stationary AP; rhs_fn(st, kc) -> moving AP;
    evac_fn(mc, st, ps) consumes the finished PSUM tile.
    """
    nc = tc.nc
    for mc in range(n_mc):
        pss = []
        for st in range(ST):
            ps = ps_pool.tile([128, TILE], FP32, tag=f"p{st}", bufs=2,
                              name=f"ps{st}")
            pss.append(ps)
        for kc in range(n_kc):
            for st in range(ST):
                nc.tensor.matmul(
                    pss[st][0:128, 0:width],
                    lhsT=lhs_fn(mc, kc),
                    rhs=rhs_fn(st, kc),
                    start=(kc == 0), stop=(kc == n_kc - 1),
                )
        for st in range(ST):
            evac_fn(mc, st, pss[st])


def _attention(tc, pools, axis, st, q_sb, kz_sb, v_sb, ot_sb, tml_sb, tmr_sb,
               abd_tiles, nch_q, nhd):
    """S + softmax + O^T for one 512-token sub-tile.

    nch_q: feature chunks of q/k (4 for w/t, 2 for h); nhd: heads (8 or 4).
    Scores psum free layout: w/t: two banks of (2 row-groups x 8 heads x 32);
    h: one bank of (4 row-groups x 4 heads x 32).
    """
    nc = tc.nc
    (xt_pool, qk_pool, v_pool, a_pool, sm_pool,
     ot_pool, y_pool, ps_pool) = pools
    GW = nhd * 32
    n_bank_groups = 512 // (2 * GW) if nhd == NH else 1  # 2-rowgroup banks

    abd_by_g = {}
    if nhd == NH:
        # w/t: 2 psum banks, each covering 2 row-groups (gh)
        for gg in range(2):
            sps = ps_pool.tile([128, 2 * GW], FP32, tag=f"p{gg}", bufs=2,
                               name="sps")
            base = axis == "t"
            if base:
                nc.tensor.matmul(
                    sps[:], lhsT=tml_sb[:], rhs=tmr_sb[:],
                    start=True, stop=False, skip_group_check=True,
                )
            nmm = 32
            i_mm = 0
            for gh in range(2):
                g = 2 * gg + gh
                for c in range(nch_q):
                    for j in range(4):
                        qcol = (g * 4 + j) * 32
                        i_mm += 1
                        nc.tensor.matmul(
                            sps[32 * j:32 * (j + 1),
                                gh * GW + 2 * c * 32:gh * GW + (2 * c + 2) * 32],
                            lhsT=q_sb[:, st, c, qcol:qcol + 32],
                            rhs=kz_sb[:, :, c,
                                      (g * 4 + j) * 32:(g * 4 + j) * 32 + 32],
                            tile_position=(0, 32 * j),
                            start=(not base),
                            stop=(base and i_mm == nmm),
                            skip_group_check=True,
                        )
            # softmax over k (free axis), one op per step per 2 row-groups
            a_sb = a_pool.tile([128, 2 * GW], BF16, tag="a")
            nc.scalar.activation(a_sb[:], sps[:],
                                 mybir.ActivationFunctionType.Exp)
            a3 = a_sb[:].rearrange("p (n k) -> p n k", n=2 * nhd)
            sums = sm_pool.tile([128, 2 * nhd], FP32, tag="sums")
            nc.vector.tensor_reduce(
                sums[:], a3, axis=mybir.AxisListType.X, op=mybir.AluOpType.add
            )
            recip = sm_pool.tile([128, 2 * nhd], FP32, tag="recip")
            nc.vector.reciprocal(recip[:], sums[:])
            nc.gpsimd.tensor_tensor(
                a3, a3,
                recip[:].unsqueeze(2).broadcast_to((128, 2 * nhd, 32)),
                mybir.AluOpType.mult,
            )
            at_sb = a_pool.tile([128, 2 * GW], BF16, tag="at")
            nc.vector.transpose(at_sb[:], a_sb[:])
            abd = abd_tiles[tc._abd_flip]
            tc._abd_flip ^= 1
            for j in range(4):
                nc.vector.tensor_copy(
                    abd[32 * j:32 * (j + 1), 512 * j:512 * (j + 1)],
                    at_sb[32 * j:32 * (j + 1), :],
                )
            abd_by_g[2 * gg] = abd
            abd_by_g[2 * gg + 1] = abd

        # O^T = V^T @ A_bd, chunk-outer
        for c in range(nch_q):
            otp = ps_pool.tile([128, TILE], FP32, name="otp",
                               tag=f"p{2 + c % 2}", bufs=2)
            for g in range(4):
                gh = g % 2
                abd4 = abd_by_g[g][:].rearrange("p (j x) -> p j x", j=4)
                for p in range(2):
                    s0 = gh * GW + (2 * c + p) * 32
                    nc.tensor.matmul(
                        otp[64 * p:64 * (p + 1), g * 128:(g + 1) * 128],
                        lhsT=v_sb[:, g, (2 * c + p) * 64:(2 * c + p + 1) * 64],
                        rhs=abd4[:, :, s0:s0 + 32],
                        tile_position=(0, 64 * p),
                    )
            if c % 2 == 0:
                nc.scalar.copy(ot_sb[:, st, c, :], otp[:])
            else:
                nc.vector.tensor_copy(ot_sb[:, st, c, :], otp[:])
    else:
        # h-pass (4 local heads): all 4 row-groups in ONE psum bank,
        # free = g*128 + head*32 + kpos; no mask, full 32-token rows.
        sps = ps_pool.tile([128, 4 * GW], FP32, tag=f"p{st % 2}", bufs=2,
                           name="sps")
        for g in range(4):
            for c in range(nch_q):
                for j in range(4):
                    qcol = (g * 4 + j) * 32
                    nc.tensor.matmul(
                        sps[32 * j:32 * (j + 1),
                            g * GW + 2 * c * 32:g * GW + (2 * c + 2) * 32],
                        lhsT=q_sb[:, st, c, qcol:qcol + 32],
                        rhs=kz_sb[:, :, c,
                                  (g * 4 + j) * 32:(g * 4 + j) * 32 + 32],
                        tile_position=(0, 32 * j),
                        start=True, stop=False,
                        skip_group_check=True,
                    )
        a_sb = a_pool.tile([128, 4 * GW], BF16, tag="a")
        nc.scalar.activation(a_sb[:], sps[:],
                             mybir.ActivationFunctionType.Exp)
        a3 = a_sb[:].rearrange("p (n k) -> p n k", n=4 * nhd)
        sums = sm_pool.tile([128, 4 * nhd], FP32, tag="sums")
        nc.vector.tensor_reduce(
            sums[:], a3, axis=mybir.AxisListType.X, op=mybir.AluOpType.add
        )
        recip = sm_pool.tile([128, 4 * nhd], FP32, tag="recip")
        nc.vector.reciprocal(recip[:], sums[:])
        nc.gpsimd.tensor_tensor(
            a3, a3,
            recip[:].unsqueeze(2).broadcast_to((128, 4 * nhd, 32)),
            mybir.AluOpType.mult,
        )
        at_sb = a_pool.tile([128, 4 * GW], BF16, tag="at")
        nc.vector.transpose(at_sb[:], a_sb[:])
        abd = abd_tiles[tc._abd_flip]
        tc._abd_flip ^= 1
        # at[32j+k, g*128 + h*32 + q] -> abd[32j+k, g*512 + j*128 + h*32 + q]
        for j in range(4):
            src = at_sb[32 * j:32 * (j + 1), :].rearrange(
                "p (g x) -> p g x", g=4)
            dst = abd[32 * j:32 * (j + 1), :].rearrange(
                "p (g x) -> p g x", g=4)[:, :, 128 * j:128 * (j + 1)]
            nc.vector.tensor_copy(dst, src)
        for c in range(NCHL):
            otp = ps_pool.tile([128, TILE], FP32, name="otp",
                               tag=f"p{2 + c % 2}", bufs=2)
            for g in range(4):
                abd4 = abd[:, 512 * g:512 * (g + 1)].rearrange(
                    "p (j x) -> p j x", j=4)
                for p in range(2):
                    s0 = (2 * c + p) * 32
                    nc.tensor.matmul(
                        otp[64 * p:64 * (p + 1), g * 128:(g + 1) * 128],
                        lhsT=v_sb[:, g, (2 * c + p) * 64:(2 * c + p + 1) * 64],
                        rhs=abd4[:, :, s0:s0 + 32],
                        tile_position=(0, 64 * p),
                    )
            if c % 2 == 0:
                nc.scalar.copy(ot_sb[:, st, c, :], otp[:])
            else:
                nc.vector.tensor_copy(ot_sb[:, st, c, :], otp[:])


def _build_pass_wt(tc, pools, axis, x_ap, w_aps, y_ap, bias_aps, tml_sb,
                   tmr_sb, kz_tiles, abd_tiles):
    """w- or t-axis pass over the core's 8192 owned tokens."""
    nc = tc.nc
    wq_sb, wk_sb, wv_sb, wo_sb = w_aps
    (xt_pool, qk_pool, v_pool, a_pool, sm_pool,
     ot_pool, y_pool, ps_pool) = pools
    ntiles = TOK_LOCAL // TILE
    y4d = y_ap.rearrange("c (t h w) -> c t h w", t=T, h=HL, w=W)

    for sup in range(ntiles // ST):
        it0 = sup * ST
        xt = xt_pool.tile([128, ST, NCH, TILE], BF16, tag="xt")
        for st in range(ST):
            for kc in range(NCH):
                nc.sync.dma_start(
                    xt[:, st, kc, :],
                    x_ap[128 * kc:128 * (kc + 1),
                         (it0 + st) * TILE:(it0 + st + 1) * TILE])

        # ---- q^T projection (feat-partition), stationary reused over ST
        q_sb = qk_pool.tile([128, ST, NCH, TILE], BF16, tag="q", bufs=1)

        def q_evac(mc, st, ps):
            if st % 2 == 0:
                nc.scalar.copy(q_sb[:, st, mc, :], ps[:])
            else:
                nc.vector.tensor_copy(q_sb[:, st, mc, :], ps[:])

        _proj_phase(tc, ps_pool, NCH, NCH,
                    lambda mc, kc: wq_sb[:, kc, 128 * mc:128 * (mc + 1)],
                    lambda st, kc: xt[:, st, kc, :], q_evac)

        # ---- k^T projection, evacuated parity-split into kz buffers
        def k_evac(mc, st, ps):
            kz = kz_tiles[st]
            if mc < 2:
                nc.scalar.copy(kz[0:64, 0, mc, :], ps[0:64, :])
                nc.scalar.copy(kz[64:128, 1, mc, :], ps[64:128, :])
            else:
                nc.vector.tensor_copy(kz[0:64, 0, mc, :], ps[0:64, :])
                nc.vector.tensor_copy(kz[64:128, 1, mc, :], ps[64:128, :])

        _proj_phase(tc, ps_pool, NCH, NCH,
                    lambda mc, kc: wk_sb[:, kc, 128 * mc:128 * (mc + 1)],
                    lambda st, kc: xt[:, st, kc, :], k_evac)

        # ---- v projection (token-partition, stationary = x) interleaved
        # with attention per sub-tile so dense v matmuls fill the PE while
        # the previous sub-tile's softmax chain runs on Scalar/Vector/GpSimd
        ot_sb = ot_pool.tile([128, ST, NCH, TILE], BF16, tag="ot", bufs=1)
        for st in range(ST):
            v_sb = v_pool.tile([128, NCH, C], BF16, tag=f"v{st}", bufs=1,
                               name=f"v{st}")
            pss = []
            for ts in range(NCH):
                ps = ps_pool.tile([128, TILE], FP32, tag=f"p{ts}", bufs=2,
                                  name="psv")
                pss.append(ps)
            for kc in range(NCH):
                for ts in range(NCH):
                    nc.tensor.matmul(
                        pss[ts][:],
                        lhsT=xt[:, st, kc, 128 * ts:128 * (ts + 1)],
                        rhs=wv_sb[:, kc, :],
                        start=(kc == 0), stop=(kc == NCH - 1),
                    )
            for ts in range(NCH):
                if ts % 2 == 0:
                    nc.scalar.copy(v_sb[:, ts, :], pss[ts][:])
                else:
                    nc.vector.tensor_copy(v_sb[:, ts, :], pss[ts][:])
            _attention(tc, pools, axis, st, q_sb, kz_tiles[st], v_sb,
                       ot_sb, tml_sb, tmr_sb, abd_tiles, NCH, NH)

        # ---- out-projection (stationary reused over ST) + y accumulate
        def y_evac(mc, st, ps):
            it = it0 + st
            cs = slice(128 * mc, 128 * (mc + 1))
            if axis == "w":
                y_sb = y_pool.tile([128, TILE], FP32, tag="yw")
                nc.scalar.activation(
                    y_sb[:], ps[:], mybir.ActivationFunctionType.Identity,
                    bias=bias_aps[mc],
                )
                nc.gpsimd.dma_start(y_ap[cs, it * TILE:(it + 1) * TILE],
                                    y_sb[:])
            else:
                # tile it covers h-row `it`; psum tokens are (w 32, t 16).
                # Evacuate transposed to (t, w), then accumulate into y via
                # a software-DGE DMA with on-the-fly add (no read-back, and
                # the psum bank frees after the copy, not the whole rmw).
                y_slice = y4d[cs, :, it, :]                   # (128, t16, w32)
                ynew = y_pool.tile([128, T, W], FP32, tag="yt2")
                yp3 = ps[:].rearrange("p (w t) -> p w t", w=W).transpose(
                    [0, 2, 1])
                if st % 2 == 0:
                    nc.scalar.copy(ynew[:], yp3)
                else:
                    nc.vector.tensor_copy(ynew[:], yp3)
                nc.gpsimd.dma_start(y_slice, ynew[:],
                                    accum_op=mybir.AluOpType.add)

        _proj_phase(tc, ps_pool, NCH, NCH,
                    lambda mc, kc: wo_sb[:, kc, 128 * mc:128 * (mc + 1)],
                    lambda st, kc: ot_sb[:, st, kc, :], y_evac)


def _build_pass_h(tc, pools, x_ap, w_aps, yh_ap, kz_tiles, abd_tiles):
    """h-axis pass: this core's 4 heads over the FULL sample; partial y_h."""
    nc = tc.nc
    wq_sb, wk_sb, wv_sb, wo_sb = w_aps
    (xt_pool, qk_pool, v_pool, a_pool, sm_pool,
     ot_pool, y_pool, ps_pool) = pools
    ntiles = TOK_FULL // TILE

    # The w/t passes leave a different nonzero footprint in abd (full
    # 512-wide strips); the h-pass layout assumes zeros outside its own
    # g*512+j*128 blocks, so re-zero before reuse.
    for t in abd_tiles:
        nc.gpsimd.memset(t[:], 0.0)

    for sup in range(ntiles // ST):
        it0 = sup * ST
        xt = xt_pool.tile([128, ST, NCH, TILE], BF16, tag="xt")
        for st in range(ST):
            for kc in range(NCH):
                nc.sync.dma_start(
                    xt[:, st, kc, :],
                    x_ap[128 * kc:128 * (kc + 1),
                         (it0 + st) * TILE:(it0 + st + 1) * TILE])

        q_sb = qk_pool.tile([128, ST, NCH, TILE], BF16, tag="q", bufs=1)

        def q_evac(mc, st, ps):
            if st % 2 == 0:
                nc.scalar.copy(q_sb[:, st, mc, :], ps[:])
            else:
                nc.vector.tensor_copy(q_sb[:, st, mc, :], ps[:])

        _proj_phase(tc, ps_pool, NCHL, NCH,
                    lambda mc, kc: wq_sb[:, kc, 128 * mc:128 * (mc + 1)],
                    lambda st, kc: xt[:, st, kc, :], q_evac)

        def k_evac(mc, st, ps):
            kz = kz_tiles[st]
            if mc == 0:
                nc.scalar.copy(kz[0:64, 0, mc, :], ps[0:64, :])
                nc.scalar.copy(kz[64:128, 1, mc, :], ps[64:128, :])
            else:
                nc.vector.tensor_copy(kz[0:64, 0, mc, :], ps[0:64, :])
                nc.vector.tensor_copy(kz[64:128, 1, mc, :], ps[64:128, :])

        _proj_phase(tc, ps_pool, NCHL, NCH,
                    lambda mc, kc: wk_sb[:, kc, 128 * mc:128 * (mc + 1)],
                    lambda st, kc: xt[:, st, kc, :], k_evac)

        ot_sb = ot_pool.tile([128, ST, NCH, TILE], BF16, tag="ot", bufs=1)
        for st in range(ST):
            v_sb = v_pool.tile([128, NCH, C], BF16, tag=f"v{st}", bufs=1,
                               name=f"v{st}")
            pss = []
            for ts in range(NCH):
                ps = ps_pool.tile([128, TILE], FP32, tag=f"p{ts}", bufs=2,
                                  name="psv")
                pss.append(ps)
            for kc in range(NCH):
                for ts in range(NCH):
                    nc.tensor.matmul(
                        pss[ts][0:128, 0:CL],
                        lhsT=xt[:, st, kc, 128 * ts:128 * (ts + 1)],
                        rhs=wv_sb[:, kc, 0:CL],
                        start=(kc == 0), stop=(kc == NCH - 1),
                    )
            for ts in range(NCH):
                if ts % 2 == 0:
                    nc.scalar.copy(v_sb[:, ts, 0:CL], pss[ts][0:128, 0:CL])
                else:
                    nc.vector.tensor_copy(v_sb[:, ts, 0:CL],
                                          pss[ts][0:128, 0:CL])
            _attention(tc, pools, "h", st, q_sb, kz_tiles[st], v_sb,
                       ot_sb, None, None, abd_tiles, NCHL, NHL)

        def y_evac(mc, st, ps):
            it = it0 + st
            cs = slice(128 * mc, 128 * (mc + 1))
            y_sb = y_pool.tile([128, TILE], BF16, tag="yh_sb")
            if mc % 2 == 0:
                nc.scalar.copy(y_sb[:], ps[:])
            else:
                nc.vector.tensor_copy(y_sb[:], ps[:])
            nc.gpsimd.dma_start(yh_ap[cs, it * TILE:(it + 1) * TILE],
                                y_sb[:])

        _proj_phase(tc, ps_pool, NCH, NCHL,
                    lambda mc, kc: wo_sb[:, kc, 128 * mc:128 * (mc + 1)],
                    lambda st, kc: ot_sb[:, st, kc, :], y_evac)


def build_program():
    """Build + compile the SPMD bass program (same program on all 8 cores)."""
    nc = bacc.Bacc(
        "TRN2", target_bir_lowering=False, debug=False,
        enable_asserts=False, num_devices=N_CORES,
    )

    def din(name, shape, dt=BF16):
        return nc.dram_tensor(name, shape, dt, kind="ExternalInput").ap()

    x_w = din("x_w", (C, TOK_LOCAL))
    x_t = din("x_t", (C, TOK_LOCAL))
    x_h = din("x_h", (C, TOK_FULL))
    w_in = {}
    for ax, cout in (("w", C), ("t", C)):
        for nm in ("wq", "wk", "wv", "wo"):
            w_in[f"{nm}_{ax}"] = din(f"{nm}_{ax}", (C, cout))
    for nm in ("wq", "wk", "wv"):
        w_in[f"{nm}_h"] = din(f"{nm}_h", (C, CL))
    w_in["wo_h"] = din("wo_h", (CL, C))
    bias_in = din("bias", (C, 1), FP32)
    tml_in = din("tml", (2, 128))
    tmr_in = din("tmr", (2, 512))
    y_ap = nc.dram_tensor("y", (C, TOK_LOCAL), FP32, kind="ExternalOutput").ap()
    yh_ap = nc.dram_tensor("y_h", (C, TOK_FULL), BF16,
                           kind="ExternalOutput").ap()

    with tile.TileContext(nc) as tc:
        with contextlib.ExitStack() as ctx:
            xt_pool = ctx.enter_context(tc.tile_pool(name="xt", bufs=2))
            w_pool = ctx.enter_context(tc.tile_pool(name="wts", bufs=2))
            qk_pool = ctx.enter_context(tc.tile_pool(name="qk", bufs=1))
            v_pool = ctx.enter_context(tc.tile_pool(name="v", bufs=1))
            a_pool = ctx.enter_context(tc.tile_pool(name="a", bufs=3))
            sm_pool = ctx.enter_context(tc.tile_pool(name="sm", bufs=3))
            ot_pool = ctx.enter_context(tc.tile_pool(name="ot", bufs=1))
            y_pool = ctx.enter_context(tc.tile_pool(name="y", bufs=3))
            ps_pool = ctx.enter_context(tc.tile_pool(name="ps", bufs=1,
                                                     space="PSUM"))
            const_pool = ctx.enter_context(tc.tile_pool(name="const", bufs=1))

            # constants
            tml_sb = const_pool.tile([2, 128], BF16)
            nc.sync.dma_start(tml_sb[:], tml_in[:])
            tmr_sb = const_pool.tile([2, 512], BF16)
            nc.sync.dma_start(tmr_sb[:], tmr_in[:])
            bias_sb = const_pool.tile([128, NCH], FP32)
            for mc in range(NCH):
                nc.sync.dma_start(
                    bias_sb[:, mc:mc + 1], bias_in[128 * mc:128 * (mc + 1), :]
                )
            bias_aps = [bias_sb[:, mc:mc + 1] for mc in range(NCH)]

            # persistent block-diagonal A^T buffers and parity-split k
            # buffers (one per sub-tile), zeroed once
            abd_tiles = []
            for i in range(2):
                t = const_pool.tile([128, 4 * 512], BF16, name=f"abd{i}")
                nc.gpsimd.memset(t[:], 0.0)
                abd_tiles.append(t)
            tc._abd_flip = 0
            kz_tiles = []
            for i in range(ST):
                t = const_pool.tile([128, 2, NCH, TILE], BF16, name=f"kz{i}")
                nc.gpsimd.memset(t[:], 0.0)
                kz_tiles.append(t)

            pools = (xt_pool, qk_pool, v_pool, a_pool, sm_pool,
                     ot_pool, y_pool, ps_pool)

            def load_w(ax, shapes):
                w_aps = []
                for nm, n_kc, ncol in shapes:
                    wt = w_pool.tile([128, NCH, C], BF16, tag=nm, name=nm)
                    for kc in range(n_kc):
                        nc.sync.dma_start(
                            wt[:, kc, 0:ncol],
                            w_in[f"{nm}_{ax}"][128 * kc:128 * (kc + 1), :],
                        )
                    w_aps.append(wt)
                return w_aps

            wt_shapes = [("wq", NCH, C), ("wk", NCH, C), ("wv", NCH, C),
                         ("wo", NCH, C)]
            h_shapes = [("wq", NCH, CL), ("wk", NCH, CL), ("wv", NCH, CL),
                        ("wo", NCHL, C)]

            w_aps = load_w("w", wt_shapes)
            _build_pass_wt(tc, pools, "w", x_w, w_aps, y_ap, bias_aps,
                           tml_sb, tmr_sb, kz_tiles, abd_tiles)
            w_aps = load_w("t", wt_shapes)
            _build_pass_wt(tc, pools, "t", x_t, w_aps, y_ap, bias_aps,
                           tml_sb, tmr_sb, kz_tiles, abd_tiles)
            w_aps = load_w("h", h_shapes)
            _build_pass_h(tc, pools, x_h, w_aps, yh_ap, kz_tiles, abd_tiles)

    nc.compile()
    return nc


_PROGRAM = None


def _get_program():
    global _PROGRAM
    if _PROGRAM is None:
        _PROGRAM = build_program()
    return _PROGRAM


def make_in_maps(inputs):
    """Host-side shard + layout prep: per-core input dicts."""
    x = np.asarray(inputs["x"], np.float32)          # (B, C, T, H, W)
    scale = 1.0 / np.sqrt(D)

    weights = {}
    for ax in ("w", "t"):
        for nm in ("wq", "wk", "wv", "wo"):
            wm = np.asarray(inputs[f"{nm}_{ax}"], np.float32)
            if nm == "wq":
                wm = wm * scale
            # lhsT layout: (C_in, C_out) = W.T
            weights[f"{nm}_{ax}"] = np.ascontiguousarray(wm.T).astype(BF16_NP)
    # h-pass: per-pair-half head slices
    h_w = {}
    for half in range(2):
        cols = slice(CL * half, CL * (half + 1))
        m = {}
        for nm in ("wq", "wk", "wv"):
            wm = np.asarray(inputs[f"{nm}_h"], np.float32)
            if nm == "wq":
                wm = wm * scale
            m[f"{nm}_h"] = np.ascontiguousarray(wm.T[:, cols]).astype(BF16_NP)
        wo = np.asarray(inputs["wo_h"], np.float32)
        m["wo_h"] = np.ascontiguousarray(wo.T[cols, :]).astype(BF16_NP)
        h_w[half] = m
    bias = (np.asarray(inputs["bo_w"], np.float32)
            + np.asarray(inputs["bo_h"], np.float32)
            + np.asarray(inputs["bo_t"], np.float32)).reshape(C, 1)

    # rank-2 additive cross-fiber mask for the t-pass:
    # S += tml.T @ tmr with tml one-hot on the query fiber and tmr = -60 on
    # cross-fiber key columns
    p = np.arange(128) % 32
    tml = np.stack([(p // 16) == e for e in range(2)]).astype(BF16_NP)
    f = np.arange(512) % 32
    tmr = np.stack([np.where((f // 16) != e, -60.0, 0.0) for e in range(2)]
                   ).astype(BF16_NP)

    in_maps = []
    for core in range(N_CORES):
        b, j = divmod(core, 2)
        xb = x[b]                                    # (C, T, H, W)
        xw = xb[:, :, 16 * j:16 * (j + 1), :]        # (C, T, HL, W) w-fastest
        xt = np.transpose(xw, (0, 2, 3, 1))          # (C, HL, W, T) t-fastest
        xh = np.transpose(xb, (0, 1, 3, 2))          # (C, T, W, H) h-fastest
        m = {
            "x_w": np.ascontiguousarray(xw).reshape(C, TOK_LOCAL).astype(BF16_NP),
            "x_t": np.ascontiguousarray(xt).reshape(C, TOK_LOCAL).astype(BF16_NP),
            "x_h": np.ascontiguousarray(xh).reshape(C, TOK_FULL).astype(BF16_NP),
            "bias": bias, "tml": tml, "tmr": tmr,
        }
        m.update(weights)
        m.update(h_w[j])
        in_maps.append(m)
    return in_maps


def assemble_output(results):
    """Gather per-core y/y_h into (B, C, T, H, W) fp32."""
    out = np.empty((B, C, T, H, W), np.float32)
    for b in range(B):
        c0, c1 = 2 * b, 2 * b + 1
        out[b, :, :, 0:HL, :] = np.asarray(results[c0]["y"]).reshape(
            C, T, HL, W)
        out[b, :, :, HL:H, :] = np.asarray(results[c1]["y"]).reshape(
            C, T, HL, W)
        yh = (np.asarray(results[c0]["y_h"]).astype(np.float32)
              + np.asarray(results[c1]["y_h"]).astype(np.float32))
        out[b] += yh.reshape(C, T, W, H).transpose(0, 1, 3, 2)
    return out


_RUNNER = None


def _get_runner():
    """Build the sharded PJRT callable once; reuse across kernel() calls."""
    global _RUNNER
    if _RUNNER is not None:
        return _RUNNER
    import jax
    from jax.sharding import Mesh, PartitionSpec
    from jax.experimental.shard_map import shard_map
    from concourse import bass2jax

    nc = _get_program()
    bass2jax.install_neuronx_cc_hook()
    partition_name = (nc.partition_id_tensor.name
                      if nc.partition_id_tensor else None)
    in_names, out_names, out_avals, zero_outs = [], [], [], []
    for alloc in nc.m.functions[0].allocations:
        if not isinstance(alloc, mybir.MemoryLocationSet):
            continue
        name = alloc.memorylocations[0].name
        if alloc.kind == "ExternalInput":
            if name != partition_name:
                in_names.append(name)
        elif alloc.kind == "ExternalOutput":
            out_names.append(name)
            shape = tuple(alloc.tensor_shape)
            dtype = mybir.dt.np(alloc.dtype)
            out_avals.append(jax.core.ShapedArray(shape, dtype))
            zero_outs.append(np.zeros((N_CORES * shape[0], *shape[1:]), dtype))
    n_params = len(in_names)
    all_in_names = list(in_names) + out_names
    if partition_name is not None:
        all_in_names.append(partition_name)

    def _body(*args):
        operands = list(args)
        if partition_name is not None:
            operands.append(bass2jax.partition_id_tensor())
        return tuple(bass2jax._bass_exec_p.bind(
            *operands,
            out_avals=tuple(out_avals),
            in_names=tuple(all_in_names),
            out_names=tuple(out_names),
            lowering_input_output_aliases=(),
            sim_require_finite=True,
            sim_require_nnan=True,
            nc=nc,
        ))

    devices = jax.devices()[:N_CORES]
    mesh = Mesh(np.asarray(devices), ("core",))
    in_specs = (PartitionSpec("core"),) * (n_params + len(out_names))
    out_specs = (PartitionSpec("core"),) * len(out_names)
    fn = jax.jit(shard_map(_body, mesh=mesh, in_specs=in_specs,
                           out_specs=out_specs, check_rep=False))

    def run(in_maps):
        concat_in = [
            np.concatenate([np.asarray(in_maps[c][nm]) for c in range(N_CORES)],
                           axis=0)
            for nm in in_names
        ]
        outs = fn(*concat_in, *zero_outs)
        return [
            {nm: np.asarray(outs[i]).reshape(N_CORES, *out_avals[i].shape)[c]
             for i, nm in enumerate(out_names)}
            for c in range(N_CORES)
        ]

    _RUNNER = run
    return run


def kernel(**inputs) -> np.ndarray:
    run = _get_runner()
    in_maps = make_in_maps(inputs)
    return assemble_output(run(in_maps))
